# revision 16
# baseline (speedup 1.0000x reference)
"""MoE DeepSeekV3 (T=2048, D=1024, E=16, I=512, topk=4, group-limited) on 8 trn2 cores.

Strategy: expert-parallel. Each core owns 2 of the 16 routed experts (weights
resident in SBUF, bf16) plus a 64-wide slice of the shared expert's inter dim.
x is replicated (fed pre-transposed + bf16 hi/lo split from the host). Each
core computes the full gate (softmax + group-limited top-4, done on-device with
a 4-term split-bf16 matmul for fp32-accurate routing), then its experts'
weighted contributions; partial outputs are summed on the host.

The gate's expert axis is permuted per-core (group-structure preserving) so
every core reads its own two experts' gate values at fixed columns 0,1 --
keeping the program SPMD across the 8 cores.
"""

import numpy as np
import ml_dtypes

T, D, E, I = 2048, 1024, 16, 512
NCORES = 8
EPC = 2            # experts per core
ISH = I // NCORES  # shared-expert inter dims per core
KD = D // 128      # contraction chunks
TCN = 4            # token chunks of 512
TTN = 4            # token tiles (128) per chunk
ITN = I // 128     # inter chunks per routed expert
BF = ml_dtypes.bfloat16

_CACHE = {}


def _build_program(unroll=1, loop_n=None):
    import concourse.bass as bass
    import concourse.tile as tile
    from concourse import bacc, mybir
    from concourse.bass import ts, ds
    from concourse.masks import make_identity

    f32 = mybir.dt.float32
    bf16 = mybir.dt.bfloat16
    AF = mybir.ActivationFunctionType
    OP = mybir.AluOpType

    nc = bacc.Bacc("TRN2", target_bir_lowering=False, debug=False,
                   enable_asserts=False, num_devices=NCORES)

    ah_d = nc.dram_tensor("ah", [D, T], bf16, kind="ExternalInput").ap()
    al_d = nc.dram_tensor("al", [D, T], bf16, kind="ExternalInput").ap()
    gh_d = nc.dram_tensor("gh", [D, E], bf16, kind="ExternalInput").ap()
    gl_d = nc.dram_tensor("gl", [D, E], bf16, kind="ExternalInput").ap()
    w1_d = nc.dram_tensor("w1t", [EPC, D, I], bf16, kind="ExternalInput").ap()
    w3_d = nc.dram_tensor("w3t", [EPC, D, I], bf16, kind="ExternalInput").ap()
    w2_d = nc.dram_tensor("w2t", [EPC, I, D], bf16, kind="ExternalInput").ap()
    ws13_d = nc.dram_tensor("ws13t", [D, 2 * ISH], bf16, kind="ExternalInput").ap()
    ws2_d = nc.dram_tensor("ws2t", [128, D], bf16, kind="ExternalInput").ap()
    y_d = nc.dram_tensor("y", [T, D], f32, kind="ExternalOutput").ap()

    with tile.TileContext(nc) as tc:
        import contextlib
        with contextlib.ExitStack() as ctx:
            consts = ctx.enter_context(tc.tile_pool(name="consts", bufs=1))
            work = ctx.enter_context(tc.tile_pool(name="work", bufs=3))
            t2p = ctx.enter_context(tc.tile_pool(name="t2p", bufs=9))
            alp = ctx.enter_context(tc.tile_pool(name="alp", bufs=4))
            hsp = ctx.enter_context(tc.tile_pool(name="hsp", bufs=2))
            ph = ctx.enter_context(tc.tile_pool(name="ph", bufs=5, space="PSUM"))
            py = ctx.enter_context(tc.tile_pool(name="py", bufs=3, space="PSUM"))

            # ---- resident tensors
            A = consts.tile([128, KD, T], bf16)
            W1 = consts.tile([128, EPC, KD, I], bf16)
            W3 = consts.tile([128, EPC, KD, I], bf16)
            W2 = consts.tile([128, EPC, ITN, D], bf16)
            WS13 = consts.tile([128, KD, 2 * ISH], bf16)
            WS2 = consts.tile([128, D], bf16)
            GH = consts.tile([128, KD, E], bf16)
            GL = consts.tile([128, KD, E], bf16)
            IDENT = consts.tile([128, 128], f32)
            HSH = consts.tile([128, T], bf16)       # shared-expert hS (rows 64+ zero)
            GBC = consts.tile([128, EPC, T], bf16)  # per-expert gate, bcast on partitions
            LT = consts.tile([16, T], f32)          # logits [e, t]
            SC = consts.tile([128, 16, E], f32)     # scores [t-part, t-tile, e]
            EXP = consts.tile([128, 16, E], f32)
            SMK = consts.tile([128, 16, E], f32)
            SEL = consts.tile([128, 16, E], f32)
            GD = consts.tile([128, 16, E], f32)     # gate_dense
            GDT = [consts.tile([1, T], bf16, name=f"gdt{el}") for el in range(EPC)]
            M1 = consts.tile([128, 16], f32)
            SM1 = consts.tile([128, 16], f32)
            RC1 = consts.tile([128, 16], f32)
            GM = consts.tile([128, 16, 4], f32)
            GM1 = consts.tile([128, 16], f32)
            EQ = consts.tile([128, 16, 4], f32)
            GM2 = consts.tile([128, 16, 4], f32)
            THR2 = consts.tile([128, 16], f32)
            GMSK = consts.tile([128, 16, 4], f32)
            T8 = consts.tile([128, 16, 8], f32)

            # ---- input DMAs (gate-critical first, split for queue parallelism)
            nc.sync.dma_start(GH[:], gh_d.rearrange("(k p) e -> p k e", p=128))
            nc.sync.dma_start(GL[:], gl_d.rearrange("(k p) e -> p k e", p=128))
            for k in range(KD):
                nc.sync.dma_start(A[:, k, 0:1024], ah_d[ts(k, 128), 0:1024])
                nc.sync.dma_start(A[:, k, 1024:2048], ah_d[ts(k, 128), 1024:2048])
            for el in range(EPC):
                nc.sync.dma_start(W1[:, el], w1_d[el].rearrange("(k p) i -> p k i", p=128))
                nc.sync.dma_start(W3[:, el], w3_d[el].rearrange("(k p) i -> p k i", p=128))
            nc.sync.dma_start(WS13[:], ws13_d.rearrange("(k p) i -> p k i", p=128))
            nc.sync.dma_start(WS2[:], ws2_d[:, :])
            for el in range(EPC):
                nc.sync.dma_start(W2[:, el], w2_d[el].rearrange("(k p) d -> p k d", p=128))
            make_identity(nc, IDENT)
            nc.vector.memset(HSH[64:128, :], 0.0)

            def emit_gate_logits():
                # 3-term split-bf16 gate: xh@gh + xl@gh + xh@gl (~fp32 accurate)
                for tcx in range(TCN):
                    gp = ph.tile([16, 512], f32, tag="h")
                    for k in range(KD):
                        nc.tensor.matmul(gp, GH[:, k, :], A[:, k, ts(tcx, 512)],
                                         start=(k == 0), stop=False)
                        nc.tensor.matmul(gp, GL[:, k, :], A[:, k, ts(tcx, 512)],
                                         start=False, stop=False)
                        alt = alp.tile([128, 512], bf16, tag="al")
                        nc.sync.dma_start(alt[:], al_d[ts(k, 128), ts(tcx, 512)])
                        nc.tensor.matmul(gp, GH[:, k, :], alt[:],
                                         start=False, stop=(k == KD - 1))
                    nc.scalar.copy(LT[:, ts(tcx, 512)], gp)

                # transpose logits to [t, e]
                for tt in range(16):
                    tp = ph.tile([128, 16], f32, tag="h")
                    nc.tensor.transpose(tp, LT[:, ts(tt, 128)], IDENT[:16, :16])
                    nc.scalar.copy(SC[:, tt, :], tp)

            def emit_softmax_topk():
                # ============ softmax over e ============
                nc.vector.reduce_max(M1[:], SC[:], axis=mybir.AxisListType.X)
                nc.vector.tensor_tensor(EXP[:], SC[:], M1[:, :, None].to_broadcast((128, 16, E)),
                                        op=OP.subtract)
                nc.scalar.activation(EXP[:], EXP[:], AF.Exp)
                nc.vector.reduce_sum(SM1[:], EXP[:], axis=mybir.AxisListType.X)
                nc.vector.reciprocal(RC1[:], SM1[:])
                nc.vector.tensor_tensor(SC[:], EXP[:], RC1[:, :, None].to_broadcast((128, 16, E)),
                                        op=OP.mult)

                # ============ group-limited top-2 groups ============
                SCg = SC[:].rearrange("p a (g e) -> p a g e", g=4)
                nc.vector.reduce_max(GM[:], SCg, axis=mybir.AxisListType.X)
                nc.vector.reduce_max(GM1[:], GM[:], axis=mybir.AxisListType.X)
                nc.vector.tensor_tensor(EQ[:], GM[:], GM1[:, :, None].to_broadcast((128, 16, 4)),
                                        op=OP.is_equal)
                nc.vector.tensor_scalar(GM2[:], EQ[:], -1e30, None, op0=OP.mult)
                nc.vector.tensor_tensor(GM2[:], GM[:], GM2[:], op=OP.add)
                nc.vector.reduce_max(THR2[:], GM2[:], axis=mybir.AxisListType.X)
                nc.vector.tensor_tensor(GMSK[:], GM[:], THR2[:, :, None].to_broadcast((128, 16, 4)),
                                        op=OP.is_ge)
                # masked scores
                nc.vector.tensor_tensor(SMK[:].rearrange("p a (g e) -> p a g e", g=4), SCg,
                                        GMSK[:, :, :, None].to_broadcast((128, 16, 4, 4)),
                                        op=OP.mult)
                # top-4 threshold per token
                for tt in range(16):
                    nc.vector.max(T8[:, tt, :], SMK[:, tt, :])
                nc.vector.tensor_tensor(SEL[:], SMK[:], T8[:, :, 3][:, :, None].to_broadcast((128, 16, E)),
                                        op=OP.is_ge)
                nc.vector.tensor_tensor(GD[:], SC[:], SEL[:], op=OP.mult)

                # transpose-back this core's two gate columns, broadcast on partitions
                for tt in range(16):
                    for el in range(EPC):
                        tp2 = ph.tile([1, 128], f32, tag="h")
                        nc.tensor.transpose(tp2, GD[:, tt, el:el + 1], IDENT[:, :])
                        nc.scalar.copy(GDT[el][:, ts(tt, 128)], tp2)
                for el in range(EPC):
                    nc.gpsimd.partition_broadcast(GBC[:, el, :], GDT[el][0:1, :])

                # ============ experts ============
                for tcx in range(TCN):
                    tsl = ts(tcx, 512)
                    # shared expert first layer (M=64 x2)
                    hs1 = ph.tile([64, 512], f32, tag="h")
                    for k in range(KD):
                        nc.tensor.matmul(hs1, WS13[:, k, 0:ISH], A[:, k, tsl],
                                         start=(k == 0), stop=(k == KD - 1))
                    hs3 = ph.tile([64, 512], f32, tag="h")
                    for k in range(KD):
                        nc.tensor.matmul(hs3, WS13[:, k, ISH:2 * ISH], A[:, k, tsl],
                                         start=(k == 0), stop=(k == KD - 1))
                    silsh = work.tile([64, 512], f32, tag="silsh")
                    nc.scalar.activation(silsh[:], hs1[:], AF.Sigmoid)
                    msh = work.tile([64, 512], f32, tag="msh")
                    nc.vector.tensor_tensor(msh[:], silsh[:], hs1[:], op=OP.mult)
                    nc.vector.tensor_tensor(HSH[0:ISH, tsl], msh[:], hs3[:], op=OP.mult)

                    # routed experts first layer
                    hs_tiles = []
                    for el in range(EPC):
                        HSe = hsp.tile([128, ITN, 512], bf16, tag=f"hs{el}")
                        hs_tiles.append(HSe)
                        for it in range(ITN):
                            h1 = ph.tile([128, 512], f32, tag="h")
                            for k in range(KD):
                                nc.tensor.matmul(h1, W1[:, el, k, ts(it, 128)], A[:, k, tsl],
                                                 start=(k == 0), stop=(k == KD - 1))
                            h3 = ph.tile([128, 512], f32, tag="h")
                            for k in range(KD):
                                nc.tensor.matmul(h3, W3[:, el, k, ts(it, 128)], A[:, k, tsl],
                                                 start=(k == 0), stop=(k == KD - 1))
                            sil = work.tile([128, 512], f32, tag="sil")
                            nc.scalar.activation(sil[:], h1[:], AF.Sigmoid)
                            t1 = work.tile([128, 512], f32, tag="t1")
                            nc.vector.tensor_tensor(t1[:], sil[:], h1[:], op=OP.mult)
                            t2 = work.tile([128, 512], f32, tag="t2")
                            nc.vector.tensor_tensor(t2[:], t1[:], h3[:], op=OP.mult)
                            nc.vector.tensor_tensor(HSe[:, it, :], t2[:], GBC[:, el, tsl],
                                                    op=OP.mult)

                    # second layer: accumulate both experts + shared into psum
                    for tt in range(TTN):
                        t0 = tcx * 512 + tt * 128
                        ystage = work.tile([128, D], f32, tag="yst")
                        for dh in range(2):
                            yp = py.tile([128, 512], f32, tag="y")
                            mm = 0
                            nmm = EPC * ITN + 1
                            for el in range(EPC):
                                for it in range(ITN):
                                    nc.tensor.matmul(yp, hs_tiles[el][:, it, ts(tt, 128)],
                                                     W2[:, el, it, ts(dh, 512)],
                                                     start=(mm == 0), stop=(mm == nmm - 1))
                                    mm += 1
                            nc.tensor.matmul(yp, HSH[:, ds(t0, 128)], WS2[:, ts(dh, 512)],
                                             start=False, stop=True)
                            nc.scalar.copy(ystage[:, ts(dh, 512)], yp)
                        nc.sync.dma_start(y_d[ds(t0, 128), :], ystage[:])

            if loop_n is not None:
                hint = (mybir.EngineType.PE, mybir.EngineType.DVE,
                        mybir.EngineType.Activation, mybir.EngineType.SP,
                        mybir.EngineType.Pool)
                with tc.For_i(0, loop_n, 1, hint_engines=hint):
                    body(0)
            else:
                for rep in range(unroll):
                    body(rep)

    nc.compile()
    return nc


def _perm_for_core(c):
    g = c // 2
    pair = [2 * c, 2 * c + 1]
    own = pair + [e for e in range(4 * g, 4 * g + 4) if e not in pair]
    rest = [e for gg in range(4) if gg != g for e in range(4 * gg, 4 * gg + 4)]
    return own + rest


def _split_bf(a):
    hi = a.astype(BF)
    lo = (a - hi.astype(np.float32)).astype(BF)
    return hi, lo


def _prep_in_maps(inputs):
    x = np.asarray(inputs["x"], np.float32)
    gate_w = np.asarray(inputs["gate_w"], np.float32)
    w1 = np.asarray(inputs["w1"], np.float32)
    w2 = np.asarray(inputs["w2"], np.float32)
    w3 = np.asarray(inputs["w3"], np.float32)
    ws1 = np.asarray(inputs["ws1"], np.float32)
    ws2 = np.asarray(inputs["ws2"], np.float32)
    ws3 = np.asarray(inputs["ws3"], np.float32)

    xh, xl = _split_bf(x)
    ah = np.ascontiguousarray(xh.T)
    al = np.ascontiguousarray(xl.T)

    in_maps = []
    for c in range(NCORES):
        perm = _perm_for_core(c)
        gwp = gate_w[perm]
        gh, gl = _split_bf(gwp)
        ghT = np.ascontiguousarray(gh.T)
        glT = np.ascontiguousarray(gl.T)
        es = [2 * c, 2 * c + 1]
        w1t = np.stack([np.ascontiguousarray(w1[e].astype(BF).T) for e in es])
        w3t = np.stack([np.ascontiguousarray(w3[e].astype(BF).T) for e in es])
        w2t = np.stack([np.ascontiguousarray(w2[e].astype(BF).T) for e in es])
        rows = np.concatenate([ws1[c * ISH:(c + 1) * ISH], ws3[c * ISH:(c + 1) * ISH]])
        ws13t = np.ascontiguousarray(rows.astype(BF).T)
        ws2t = np.zeros((128, D), BF)
        ws2t[:ISH] = ws2[:, c * ISH:(c + 1) * ISH].T.astype(BF)
        in_maps.append({
            "ah": ah, "al": al, "gh": ghT, "gl": glT,
            "w1t": w1t, "w3t": w3t, "w2t": w2t,
            "ws13t": ws13t, "ws2t": ws2t,
        })
    return in_maps


def get_program(unroll=1, loop_n=None):
    key = ("nc", unroll, loop_n)
    if key not in _CACHE:
        _CACHE[key] = _build_program(unroll, loop_n)
    return _CACHE[key]


def run_on_device(inputs, unroll=1, loop_n=None):
    from concourse import bass_utils
    nc = get_program(unroll, loop_n)
    in_maps = _prep_in_maps(inputs)
    res = bass_utils.run_bass_kernel_spmd(nc, in_maps, core_ids=list(range(NCORES)))
    return res


def kernel(**inputs) -> np.ndarray:
    res = run_on_device(inputs)
    y = np.zeros((T, D), np.float32)
    for c in range(NCORES):
        y += res.results[c]["y"]
    return y
